# revision 9
# baseline (speedup 1.0000x reference)
"""TRN2 Bass kernel for nn_CoreAttention_34875134444341 (v5).

Strategy (8 NeuronCores, pairwise AllGather):
  - Data-parallel over batch (4) x causal-balanced query-row split (2).
  - Each core computes K/V projections ONLY for its own 1024 query
    tokens, then a 2-core AllGather (replica groups [0,1],[2,3],[4,5],
    [6,7]) exchanges K/V shards, overlapped with the Q projections.
  - All matmul operands fp16 (same PE rate as bf16, 8x finer mantissa).
  - Softmax denominators accumulated on the Vector engine (fp16 adds of
    the exp tiles) + ONE ones-stationary matmul per (slot, kvh) instead
    of per-group sum matmuls: removes ~55us of TensorE work.
  - Causal masking is multiplicative post-exp on the Vector engine
    (bankdiag tile + per-slot per-bank flags), not identity matmuls.
  - W_o is interleaved INTO the attention slot loop at matmul
    granularity (credit-paced filler) so TensorE never stalls on the
    exp (Scalar) chain; output rows stream out as slots complete.
"""

import sys
from collections import deque

sys.path.insert(0, "/opt/trn_rl_repo")

import numpy as np

B, S, D = 4, 2048, 2048
H, HKV, DK = 16, 4, 128
RQ = RKV = 512
GROUP = H // HKV
P = 128

TILE_R = 128  # query rows per slot
KB = 128  # keys per block
NG = [8, 7, 6, 5, 4, 3, 2, 1]  # key-block pairs per slot (both parities)
TILE_ASSIGN = {
    0: [15, 12, 11, 8, 7, 4, 3, 0],
    1: [14, 13, 10, 9, 6, 5, 2, 1],
}  # slot -> query tile (descending); tiles double as this core's kv shard

ROWS_PER_CORE = 8 * TILE_R  # 1024
SHARD = 8 * TILE_R  # kv tokens per core
SLOT_ORDER = [7, 6, 5, 4, 3, 2, 1, 0]  # ascending ng
CREDIT_PER_GROUP = 2.2  # wo filler matmuls per attention group

_CACHE = {}
TRACE = False
LAST_RESULT = None

F16 = np.float16


def _rows_sched(parity):
    return np.concatenate(
        [np.arange(t * TILE_R, (t + 1) * TILE_R) for t in TILE_ASSIGN[parity]]
    )


def _make_bankdiag(parity):
    """[128 key, 2 banks, 512 (4 heads x 128 rows)] multiplicative mask.

    Own bank (bank == parity) gets the causal diagonal keep-mask (key <=
    row), tiled over the 4 heads; the other bank gets all-ones. Applied
    only on the LAST group of each (slot, kvh) unit.
    """
    diag_keep = (
        np.arange(P)[:, None] <= np.arange(TILE_R)[None, :]
    ).astype(np.float32)
    diag4 = np.tile(diag_keep, (1, 4))
    m = np.ones((P, 2, 512), np.float32)
    m[:, parity] = diag4
    return m.astype(F16)


def _make_flags(parity):
    """[128, 8 slots, 2 banks]: 0 kills a bank's last-group block when the
    OTHER rank's schedule is one block short of the uniform NG schedule."""
    own, other = parity, 1 - parity
    own_tiles = sorted(TILE_ASSIGN[own])
    other_tiles = sorted(TILE_ASSIGN[other])
    f = np.ones((P, 8, 2), np.float32)
    for s in range(8):
        t = TILE_ASSIGN[parity][s]
        ng = NG[s]
        assert sum(1 for x in own_tiles if x <= t) == ng
        exact_other = sum(1 for x in other_tiles if x <= t)
        assert exact_other in (ng, ng - 1)
        if exact_other == ng - 1:
            f[:, s, other] = 0.0
    return f.astype(np.float32)


def _build_nc():
    import concourse.tile as tile
    from concourse import bacc, mybir

    f32 = mybir.dt.float32
    f16 = mybir.dt.float16
    Exp = mybir.ActivationFunctionType.Exp
    Mult = mybir.AluOpType.mult
    Add = mybir.AluOpType.add

    nc = bacc.Bacc("TRN2", target_bir_lowering=False, debug=False, num_devices=8)

    xTq = nc.dram_tensor("xTq", [D, ROWS_PER_CORE], f16, kind="ExternalInput")
    wq1 = nc.dram_tensor("wq1", [D, RQ], f16, kind="ExternalInput")
    wq2 = nc.dram_tensor("wq2", [RQ, H * DK], f16, kind="ExternalInput")
    wk1 = nc.dram_tensor("wk1", [D, RKV], f16, kind="ExternalInput")
    wk2 = nc.dram_tensor("wk2", [RKV, HKV * DK], f16, kind="ExternalInput")
    wv1 = nc.dram_tensor("wv1", [D, RKV], f16, kind="ExternalInput")
    wv2 = nc.dram_tensor("wv2", [RKV, HKV * DK], f16, kind="ExternalInput")
    wo = nc.dram_tensor("wo", [D, D], f16, kind="ExternalInput")
    bankdiag_in = nc.dram_tensor("bankdiag_in", [P, 2, 512], f16, kind="ExternalInput")
    flags_in = nc.dram_tensor("flags_in", [P, 8, 2], f32, kind="ExternalInput")
    ones_in = nc.dram_tensor("ones_in", [P, P], f16, kind="ExternalInput")
    out = nc.dram_tensor("out", [ROWS_PER_CORE, D], f32, kind="ExternalOutput")

    xTq_t = xTq.rearrange("(dc p) r -> p dc r", p=P)  # [128, 16, 1024]
    wq1_t = wq1.rearrange("(dc p) r -> p dc r", p=P)  # [128, 16, 512]
    wk1_t = wk1.rearrange("(dc p) r -> p dc r", p=P)
    wv1_t = wv1.rearrange("(dc p) r -> p dc r", p=P)
    wq2_t = wq2.rearrange("(rc p) h -> p rc h", p=P)  # [128, 4, 2048]
    wk2_t = wk2.rearrange("(rc p) h -> p rc h", p=P)  # [128, 4, 512]
    wv2_t = wv2.rearrange("(rc p) h -> p rc h", p=P)
    wo_t = wo.rearrange("(hc p) o -> p hc o", p=P)  # [128, 16, 2048]

    with tile.TileContext(nc) as tc:
        with (
            tc.tile_pool(name="keep", bufs=1) as keep,
            tc.tile_pool(name="q1keep", bufs=1) as q1keep,
            tc.tile_pool(name="cc_dram", bufs=1, space="DRAM") as cc_dram,
        ):
            ones_sb = keep.tile([P, P], f16)
            bankdiag_sb = keep.tile([P, 2, 512], f16)
            flags_sb = keep.tile([P, 8, 2], f32)
            kT_sb = keep.tile([P, HKV, S], f16)
            v_sb = keep.tile([P, S // P, HKV * DK], f16)
            qT_all = keep.tile([P, H, ROWS_PER_CORE], f16)
            q1t = q1keep.tile([P, 4, ROWS_PER_CORE], f16)

            cc_ins = [
                cc_dram.tile([P, 4096], f16, name=f"cc_in_{c}") for c in range(2)
            ]
            cc_outs = [
                cc_dram.tile([2, P, 4096], f16, name=f"cc_out_{c}")
                for c in range(2)
            ]

            # =========== Phase A: K/V projections + Q1 ====================
            with tc.tile_pool(name="proj_a", bufs=1) as proj_a:
                wq1_ts = [
                    proj_a.tile([P, 4, RQ], f16, name=f"wq1_p{dq}")
                    for dq in range(4)
                ]
                # xtq keyed (dq, cn): [128, 4, 512] halves so the first
                # matmul's dependencies land in ~2us instead of 15us
                xtq_ts = {
                    (dq, cn): proj_a.tile(
                        [P, 4, 512], f16, name=f"xtq_p{dq}_c{cn}"
                    )
                    for dq in range(4)
                    for cn in range(2)
                }

                with tc.tile_pool(name="kv_sc", bufs=1) as kv_sc:
                    w1k_sb = kv_sc.tile([P, 16, RKV], f16)
                    w1v_sb = kv_sc.tile([P, 16, RKV], f16)
                    w2k_sb = kv_sc.tile([P, 4, HKV * DK], f16)
                    w2v_sb = kv_sc.tile([P, 4, HKV * DK], f16)
                    midk = kv_sc.tile([P, 4, ROWS_PER_CORE], f16)
                    midv = kv_sc.tile([P, 4, ROWS_PER_CORE], f16)

                    # ---- DMA order = first-dependency-first ----
                    for dc in range(4):
                        nc.sync.dma_start(
                            w1k_sb[:, dc : dc + 1], wk1_t[:, dc : dc + 1]
                        )
                        nc.sync.dma_start(
                            xtq_ts[(0, 1)][:, dc : dc + 1],
                            xTq_t[:, dc : dc + 1, 512:1024],
                        )
                    for dq in range(1, 4):
                        nc.sync.dma_start(
                            w1k_sb[:, dq * 4 : (dq + 1) * 4],
                            wk1_t[:, dq * 4 : (dq + 1) * 4],
                        )
                        nc.sync.dma_start(
                            xtq_ts[(dq, 1)][:],
                            xTq_t[:, dq * 4 : (dq + 1) * 4, 512:1024],
                        )
                    for dq in range(4):
                        nc.sync.dma_start(
                            w1v_sb[:, dq * 4 : (dq + 1) * 4],
                            wv1_t[:, dq * 4 : (dq + 1) * 4],
                        )
                    nc.sync.dma_start(w2k_sb[:], wk2_t)
                    nc.sync.dma_start(w2v_sb[:], wv2_t)
                    for dq in range(4):
                        nc.sync.dma_start(
                            xtq_ts[(dq, 0)][:],
                            xTq_t[:, dq * 4 : (dq + 1) * 4, 0:512],
                        )
                    for dq in range(4):
                        nc.sync.dma_start(
                            wq1_ts[dq][:], wq1_t[:, dq * 4 : (dq + 1) * 4]
                        )
                    nc.sync.dma_start(ones_sb[:], ones_in[:])
                    nc.sync.dma_start(bankdiag_sb[:], bankdiag_in[:])
                    nc.sync.dma_start(flags_sb[:], flags_in[:])

                    with (
                        tc.tile_pool(name="kv_bounce", bufs=6) as kv_bounce,
                        tc.tile_pool(name="kv_ps1", bufs=4, space="PSUM") as kv_ps1,
                        tc.tile_pool(name="kv_ps2", bufs=4, space="PSUM") as kv_ps2,
                    ):
                        # HAM warmup: junk matmuls while the head DMAs land
                        warm = kv_bounce.tile([P, 512], f16, tag="warm", bufs=1)
                        nc.gpsimd.memset(warm[:], 0.0)
                        for w in range(12):
                            ps_w = kv_ps1.tile(
                                [P, 512], f32, tag="ps1", name=f"ps_w{w}"
                            )
                            nc.tensor.matmul(
                                ps_w[:], warm[:, 0:P], warm[:],
                                start=True, stop=True,
                            )

                        def kv_mids(which, cn, piece_major):
                            w1_sb = w1k_sb if which == 0 else w1v_sb
                            mid = midk if which == 0 else midv
                            lo = cn * 512
                            if piece_major:
                                ps_ks = [
                                    kv_ps1.tile(
                                        [P, 512], f32, tag="ps1", name=f"ps_k_{rc}"
                                    )
                                    for rc in range(4)
                                ]
                                for dq in range(4):
                                    for rc in range(4):
                                        for dc in range(4 * dq, 4 * dq + 4):
                                            nc.tensor.matmul(
                                                ps_ks[rc][:],
                                                w1_sb[:, dc, rc * P : (rc + 1) * P],
                                                xtq_ts[(dc // 4, cn)][:, dc % 4],
                                                start=(dc == 0),
                                                stop=(dc == 15),
                                            )
                                for rc in range(4):
                                    nc.any.tensor_copy(
                                        mid[:, rc, lo : lo + 512], ps_ks[rc][:]
                                    )
                            else:
                                for rc in range(4):
                                    ps_1 = kv_ps1.tile([P, 512], f32, tag="ps1")
                                    for dc in range(16):
                                        nc.tensor.matmul(
                                            ps_1[:],
                                            w1_sb[:, dc, rc * P : (rc + 1) * P],
                                            xtq_ts[(dc // 4, cn)][:, dc % 4],
                                            start=(dc == 0),
                                            stop=(dc == 15),
                                        )
                                    nc.any.tensor_copy(
                                        mid[:, rc, lo : lo + 512], ps_1[:]
                                    )

                        def kv_gemm2(cn, cc_in):
                            lo = cn * 512
                            for hc in range(HKV):
                                ps_2 = kv_ps2.tile([P, 512], f32, tag="ps2")
                                for rc in range(4):
                                    nc.tensor.matmul(
                                        ps_2[:],
                                        w2k_sb[:, rc, hc * P : (hc + 1) * P],
                                        midk[:, rc, lo : lo + 512],
                                        start=(rc == 0),
                                        stop=(rc == 3),
                                    )
                                kb = kv_bounce.tile([P, 512], f16, tag="kb")
                                nc.any.tensor_copy(kb[:], ps_2[:])
                                for h2 in range(2):
                                    nc.sync.dma_start(
                                        cc_in[
                                            :,
                                            hc * 512
                                            + h2 * 256 : hc * 512
                                            + (h2 + 1) * 256,
                                        ],
                                        kb[:, h2 * 256 : (h2 + 1) * 256],
                                    )
                            for i in range(4):
                                ps_2 = kv_ps2.tile([P, 512], f32, tag="ps2")
                                for rc in range(4):
                                    nc.tensor.matmul(
                                        ps_2[:],
                                        midv[:, rc, lo + i * P : lo + (i + 1) * P],
                                        w2v_sb[:, rc],
                                        start=(rc == 0),
                                        stop=(rc == 3),
                                    )
                                vb = kv_bounce.tile([P, 512], f16, tag="vb")
                                nc.any.tensor_copy(vb[:], ps_2[:])
                                for h2 in range(2):
                                    nc.sync.dma_start(
                                        cc_in[
                                            :,
                                            2048
                                            + i * 512
                                            + h2 * 256 : 2048
                                            + i * 512
                                            + (h2 + 1) * 256,
                                        ],
                                        vb[:, h2 * 256 : (h2 + 1) * 256],
                                    )

                        for ci, cn in enumerate((1, 0)):
                            kv_mids(0, cn, piece_major=(ci == 0))
                            kv_mids(1, cn, piece_major=False)
                            kv_gemm2(cn, cc_ins[ci])
                            nc.gpsimd.collective_compute(
                                "AllGather",
                                mybir.AluOpType.bypass,
                                replica_groups=[[0, 1], [2, 3], [4, 5], [6, 7]],
                                ins=[cc_ins[ci][:].opt()],
                                outs=[cc_outs[ci][:].opt()],
                            )

                # K/V loadbacks (gated on the AllGathers) run during Q1
                for ci in range(2):
                    off = 512 if ci == 0 else 0
                    voff = 4 if ci == 0 else 0
                    for r in range(2):
                        for kvh in range(HKV):
                            nc.sync.dma_start(
                                kT_sb[
                                    :,
                                    kvh,
                                    r * SHARD + off : r * SHARD + off + 512,
                                ],
                                cc_outs[ci][r, :, kvh * 512 : (kvh + 1) * 512],
                            )
                        for i in range(4):
                            nc.sync.dma_start(
                                v_sb[:, r * 8 + voff + i, :],
                                cc_outs[ci][
                                    r, :, 2048 + i * 512 : 2048 + (i + 1) * 512
                                ],
                            )

                # ------- Q1 projection (resident) -------------------------
                with tc.tile_pool(name="q1_ps", bufs=4, space="PSUM") as q1_ps:
                    for cn in (1, 0):
                        for rc in range(4):
                            ps_q = q1_ps.tile([P, 512], f32, tag="psq1")
                            for dc in range(16):
                                nc.tensor.matmul(
                                    ps_q[:],
                                    wq1_ts[dc // 4][:, dc % 4, rc * P : (rc + 1) * P],
                                    xtq_ts[(dc // 4, cn)][:, dc % 4],
                                    start=(dc == 0),
                                    stop=(dc == 15),
                                )
                            nc.any.tensor_copy(
                                q1t[:, rc, cn * 512 : cn * 512 + 512], ps_q[:]
                            )

            # =========== Phase B: Q2 + attention + interleaved Wo =========
            with tc.tile_pool(name="wo_w", bufs=1) as wo_w:
                # Q2 weights first (needed immediately), then CC loadbacks
                # (needed at attention start, ~28us out), then Wo weights
                # (first needed ~10us into attention).
                with (
                    tc.tile_pool(name="q2_w", bufs=1) as q2_w,
                    tc.tile_pool(name="q2_ps", bufs=4, space="PSUM") as q2_ps,
                ):
                    wq2_sb = q2_w.tile([P, 4, H * DK], f16)
                    for hq in range(4):
                        nc.sync.dma_start(
                            wq2_sb[:, :, hq * 512 : (hq + 1) * 512],
                            wq2_t[:, :, hq * 512 : (hq + 1) * 512],
                        )
                    wq2_tail = wo_w.tile([P, 4, 512], f16)
                    nc.sync.dma_start(wq2_tail[:], wq2_t[:, :, 1536:2048])
                    wo_tiles = []
                    for oc in range(4):
                        t = wo_w.tile([P, 16, 512], f16, name=f"wo_c{oc}")
                        for half in range(2):
                            nc.sync.dma_start(
                                t[:, half * 8 : (half + 1) * 8],
                                wo_t[
                                    :,
                                    half * 8 : (half + 1) * 8,
                                    oc * 512 : (oc + 1) * 512,
                                ],
                            )
                        wo_tiles.append(t)

                    for cn in (1, 0):
                        for h in range(H):
                            if cn == 0 and h >= 12:
                                continue  # deferred as attention-start filler
                            ps_qT = q2_ps.tile([P, 512], f32, tag="psq2")
                            for rc in range(4):
                                nc.tensor.matmul(
                                    ps_qT[:],
                                    wq2_sb[:, rc, h * P : (h + 1) * P],
                                    q1t[:, rc, cn * 512 : cn * 512 + 512],
                                    start=(rc == 0),
                                    stop=(rc == 3),
                                )
                            nc.any.tensor_copy(
                                qT_all[:, h, cn * 512 : cn * 512 + 512],
                                ps_qT[:],
                            )

                # --------- attention + Wo filler --------------------------
                with (
                    tc.tile_pool(name="at_e", bufs=6) as at_e,
                    tc.tile_pool(name="at_acc", bufs=2) as at_accp,
                    tc.tile_pool(name="at_rec", bufs=3) as at_rec,
                    tc.tile_pool(name="at_attn", bufs=6) as at_attn,
                    tc.tile_pool(name="wo_out", bufs=4) as wo_out,
                    tc.tile_pool(name="ps_sc", bufs=3, space="PSUM") as ps_scp,
                    tc.tile_pool(name="ps_at", bufs=2, space="PSUM") as ps_atp,
                    tc.tile_pool(name="ps_sum", bufs=1, space="PSUM") as ps_sump,
                    tc.tile_pool(name="ps_o", bufs=2, space="PSUM") as ps_op,
                ):
                    attn_tiles = {}

                    class WoFiller:
                        """Emits individual Wo matmuls as tensor-stream
                        filler, paced by attention-group credits."""

                        def __init__(self):
                            self.extra = deque()  # closures, drained first
                            self.items = deque()  # (rc, oc) psum groups
                            self.cur = None  # [rc, oc, hc, ps]
                            self.credit = 0.0

                        def push_slot(self, rc):
                            for oc in range(4):
                                self.items.append((rc, oc))

                        def _emit_one(self):
                            if self.extra:
                                self.extra.popleft()()
                                return True
                            if self.cur is None:
                                if not self.items:
                                    return False
                                rc, oc = self.items.popleft()
                                ps = ps_op.tile(
                                    [P, 512], f32, tag="o",
                                    name=f"ps_o_{rc}_{oc}",
                                )
                                self.cur = [rc, oc, 0, ps]
                            rc, oc, hc, ps = self.cur
                            at = attn_tiles[rc]
                            nc.tensor.matmul(
                                ps[:],
                                at[:, hc // 4, (hc % 4) * P : (hc % 4 + 1) * P],
                                wo_tiles[oc][:, hc],
                                start=(hc == 0),
                                stop=(hc == 15),
                            )
                            if hc == 15:
                                o_sb = wo_out.tile(
                                    [P, 512], f32, tag="osb",
                                    name=f"o_sb_{rc}_{oc}",
                                )
                                nc.vector.tensor_copy(o_sb[:], ps[:])
                                for hf in range(2):
                                    nc.sync.dma_start(
                                        out[
                                            rc * P : (rc + 1) * P,
                                            oc * 512
                                            + hf * 256 : oc * 512
                                            + (hf + 1) * 256,
                                        ],
                                        o_sb[:, hf * 256 : (hf + 1) * 256],
                                    )
                                self.cur = None
                            else:
                                self.cur[2] = hc + 1
                            return True

                        def fill(self):
                            self.credit += CREDIT_PER_GROUP
                            while self.credit >= 1.0:
                                if not self._emit_one():
                                    self.credit = 0.0
                                    return
                                self.credit -= 1.0

                        def drain(self):
                            while self._emit_one():
                                pass

                    filler = WoFiller()
                    q2t_state = {}

                    def mk_q2_tail(h, rc):
                        def f():
                            if rc == 0:
                                q2t_state[h] = ps_op.tile(
                                    [P, 512], f32, tag="o", name=f"ps_q2t_{h}"
                                )
                            ps = q2t_state[h]
                            nc.tensor.matmul(
                                ps[:],
                                wq2_tail[:, rc, (h - 12) * P : (h - 11) * P],
                                q1t[:, rc, 0:512],
                                start=(rc == 0),
                                stop=(rc == 3),
                            )
                            if rc == 3:
                                nc.vector.tensor_copy(
                                    qT_all[:, h, 0:512], ps[:]
                                )
                        return f

                    for h in range(12, 16):
                        for rc in range(4):
                            filler.extra.append(mk_q2_tail(h, rc))
                    pending = deque()  # deferred (kvh-unit) softmax flushes

                    def do_flush(item):
                        s, kvh, acc, ps_at, attn_s = item
                        ps_sum = ps_sump.tile(
                            [P, 512], f32, tag="sum", name=f"ps_sum_{s}_{kvh}"
                        )
                        nc.tensor.matmul(
                            ps_sum[:], ones_sb[:], acc[:], start=True, stop=True
                        )
                        rec = at_rec.tile(
                            [P, 512], f32, tag="rec", name=f"rec_{s}_{kvh}"
                        )
                        nc.vector.reciprocal_approx_fast(out=rec[:], in_=ps_sum[:])
                        nc.vector.tensor_tensor(
                            attn_s[:, kvh], ps_at[:], rec[:], Mult
                        )

                    for si, s in enumerate(SLOT_ORDER):
                        ng = NG[s]
                        attn_s = at_attn.tile(
                            [P, HKV, 512], f16, tag="attn", name=f"attn_{s}"
                        )
                        attn_tiles[s] = attn_s
                        for kvh in range(HKV):
                            h0 = 4 * kvh
                            ps_at = ps_atp.tile(
                                [P, 512], f32, tag="at", name=f"ps_at_{s}_{kvh}"
                            )
                            acc = at_accp.tile(
                                [P, 512], f16, tag="acc", name=f"acc_{s}_{kvh}"
                            )
                            sc = {}

                            def emit_qk(g, sc=sc, s=s, kvh=kvh, h0=h0):
                                pos = 7 - g
                                ts = []
                                for j in range(2):
                                    t = ps_scp.tile(
                                        [P, 512], f32, tag="sc",
                                        name=f"sc_{s}_{kvh}_{g}_{j}",
                                    )
                                    nc.tensor.matmul(
                                        t[:],
                                        kT_sb[
                                            :,
                                            kvh,
                                            j * SHARD
                                            + pos * KB : j * SHARD
                                            + (pos + 1) * KB,
                                        ],
                                        qT_all[
                                            :,
                                            h0 : h0 + 4,
                                            s * TILE_R : (s + 1) * TILE_R,
                                        ],
                                        start=True,
                                        stop=True,
                                    )
                                    ts.append(t)
                                sc[g] = ts

                            emit_qk(0)
                            for g in range(ng):
                                pos = 7 - g
                                last = g == ng - 1
                                t0, t1 = sc.pop(g)
                                e_sb = at_e.tile(
                                    [P, 2, 512], f16, tag="e",
                                    name=f"e_{s}_{kvh}_{g}",
                                )
                                nc.scalar.activation(e_sb[:, 0], t0[:], Exp)
                                nc.scalar.activation(e_sb[:, 1], t1[:], Exp)
                                if last:
                                    nc.vector.tensor_tensor(
                                        e_sb[:], e_sb[:], bankdiag_sb[:], Mult
                                    )
                                    for j in range(2):
                                        nc.vector.tensor_scalar_mul(
                                            e_sb[:, j], e_sb[:, j],
                                            flags_sb[:, s, j : j + 1],
                                        )
                                if g == 0:
                                    nc.vector.tensor_tensor(
                                        acc[:], e_sb[:, 0], e_sb[:, 1], Add
                                    )
                                else:
                                    nc.vector.tensor_tensor(
                                        acc[:], acc[:], e_sb[:, 0], Add
                                    )
                                    nc.vector.tensor_tensor(
                                        acc[:], acc[:], e_sb[:, 1], Add
                                    )
                                if not last:
                                    emit_qk(g + 1)
                                filler.fill()
                                for j in range(2):
                                    nc.tensor.matmul(
                                        ps_at[:],
                                        v_sb[
                                            :,
                                            j * 8 + pos,
                                            kvh * DK : (kvh + 1) * DK,
                                        ],
                                        e_sb[:, j],
                                        start=(g == 0 and j == 0),
                                        stop=(last and j == 1),
                                    )
                            pending.append((s, kvh, acc, ps_at, attn_s))
                            if len(pending) > 1:
                                do_flush(pending.popleft())
                            if kvh == 2 and si > 0:
                                filler.push_slot(SLOT_ORDER[si - 1])
                    while pending:
                        do_flush(pending.popleft())
                    filler.push_slot(SLOT_ORDER[-1])
                    filler.drain()

    nc.finalize()
    return nc


def kernel(x, Wq1, Wq2, Wk1, Wk2, Wv1, Wv2, Wo):
    global LAST_RESULT
    from concourse.bass_utils import run_bass_kernel_spmd

    x = np.asarray(x, dtype=np.float32)
    Wq1 = np.asarray(Wq1, dtype=np.float32)
    Wq2 = np.asarray(Wq2, dtype=np.float32)
    Wk1 = np.asarray(Wk1, dtype=np.float32)
    Wk2 = np.asarray(Wk2, dtype=np.float32)
    Wv1 = np.asarray(Wv1, dtype=np.float32)
    Wv2 = np.asarray(Wv2, dtype=np.float32)
    Wo = np.asarray(Wo, dtype=np.float32)

    if "nc" not in _CACHE:
        _CACHE["nc"] = _build_nc()
    nc = _CACHE["nc"]

    wq1_h = Wq1.astype(F16)
    wq2_h = (Wq2 / np.sqrt(DK)).astype(F16)
    wk1_h = Wk1.astype(F16)
    wk2_h = Wk2.astype(F16)
    wv1_h = Wv1.astype(F16)
    wv2_h = Wv2.astype(F16)
    wo_h = Wo.astype(F16)
    bankdiags = {p: _make_bankdiag(p) for p in range(2)}
    flags = {p: _make_flags(p) for p in range(2)}
    rows = {p: _rows_sched(p) for p in range(2)}
    ones_np = np.ones((P, P), F16)

    xT_h = {}
    for batch in range(B):
        xT_h[batch] = np.ascontiguousarray(x[batch].T).astype(F16)

    in_maps = []
    for core in range(8):
        batch, parity = core // 2, core % 2
        xT = xT_h[batch]
        in_maps.append(
            {
                "xTq": np.ascontiguousarray(xT[:, rows[parity]]),
                "wq1": wq1_h,
                "wq2": wq2_h,
                "wk1": wk1_h,
                "wk2": wk2_h,
                "wv1": wv1_h,
                "wv2": wv2_h,
                "wo": wo_h,
                "bankdiag_in": bankdiags[parity],
                "flags_in": flags[parity],
                "ones_in": ones_np,
            }
        )

    res = run_bass_kernel_spmd(nc, in_maps, core_ids=list(range(8)), trace=TRACE)
    LAST_RESULT = res

    out_full = np.empty((B, S, D), np.float32)
    for core in range(8):
        batch, parity = core // 2, core % 2
        out_full[batch][rows[parity]] = res.results[core]["out"]
    return out_full
